# revision 31
# baseline (speedup 1.0000x reference)
# Trainium2 Bass kernel for nn_Attention_60464549593105.
#
# Math (per batch b, spatial point (h,w), seq s):
#   energy[k] = tanh( We @ enc[:,s] + Wh @ hidden + b_att )      (K=128)
#   score[s]  = W_v . energy
#   out[s]    = softmax_s(score)
#
# Strategy: shard the H axis across 8 cores (8 rows each) so softmax over
# seq is core-local (no collectives). Each core streams its 64 MiB slice of
# encoder_outputs once -> memory-bound; measured ~233 us vs the ~190 us
# HBM-stream floor (358 GB/s per core):
#   - all matmuls run in float32r (~1e-4/element rounding; measured 1.6e-4
#     rel on a 128-deep contraction vs bf16's 2.3e-3) which streams at
#     1 cycle/row vs fp32's LOW_HIGH 4 cycles/row. The fp32->fp32r rounding
#     is free everywhere: SWDGE DMA casts during the HBM load, and ACT
#     writes its tanh/exp outputs as fp32r directly.
#   - proj_e: PE matmul lhsT=We^T [E,K], rhs=enc chunk [E, 512] -> psum
#   - +proj_h: mostly a DVE add of a precomputed proj_h tile; 1 in 8 tiles
#     uses a second accumulating PE matmul to balance PE vs DVE occupancy
#   - tanh(+b_att): one ACT pass per s (psum -> sbuf fp32r), bias = b_att
#   - scores: matvec with a sliding-window masked W_v stationary operand so
#     per-s results accumulate directly into [32, 512] psum tiles
#     (partition = s mod 32) -> softmax-ready layout; two 32-row groups per
#     batch so the first exp can start halfway through the batch
#   - softmax over s: exp twice (fp32r copy for the sum matmul, fp32 copy
#     for the numerator; max-subtraction skipped -- |score| <= sum|W_v| ~ 5
#     so exp is safe in fp32), all-ones [32,64] matmuls = sum-over-
#     partitions broadcast to all 64 rows, DVE reciprocal + multiply.
#   - the last batch's DMA chunks taper off so the pipeline drains quickly.

import numpy as np

B, D, E, S, H, W = 4, 128, 128, 64, 64, 64
K = 128
NCORES = 8
HSH = H // NCORES          # h rows per core
FREE = HSH * W             # free-dim elements per (b, s) tile
SCH = 24                   # seq positions per enc DMA chunk (6 MiB per DMA)
ENC_BUFS = 3
PE_ADD_EVERY = 8           # s % PE_ADD_EVERY == 0 -> proj_h via PE, else DVE

_CACHE = {}


def _build_bass():
    import concourse.bacc as bacc
    import concourse.mybir as mybir
    import concourse.tile as tile
    from contextlib import ExitStack

    f32 = mybir.dt.float32
    f32r = mybir.dt.float32r
    AF = mybir.ActivationFunctionType

    nc = bacc.Bacc("TRN2", target_bir_lowering=False, debug=False)
    enc = nc.dram_tensor("enc", [B, E, S * FREE], f32, kind="ExternalInput")
    hid = nc.dram_tensor("hid", [B, D, FREE], f32, kind="ExternalInput")
    weT = nc.dram_tensor("weT", [E, K], f32, kind="ExternalInput")
    whT = nc.dram_tensor("whT", [D, K], f32, kind="ExternalInput")
    batt = nc.dram_tensor("batt", [K, 1], f32, kind="ExternalInput")
    wvs = nc.dram_tensor("wvs", [K, 2 * S], f32, kind="ExternalInput")
    out = nc.dram_tensor("out", [B, S, FREE], f32, kind="ExternalOutput")

    with tile.TileContext(nc) as tc, ExitStack() as ctx:
        consts = ctx.enter_context(tc.tile_pool(name="consts", bufs=1))
        encp = ctx.enter_context(tc.tile_pool(name="encp", bufs=ENC_BUFS))
        epsum = ctx.enter_context(tc.tile_pool(name="epsum", bufs=5, space="PSUM"))
        spsum = ctx.enter_context(tc.tile_pool(name="spsum", bufs=2, space="PSUM"))
        work = ctx.enter_context(tc.tile_pool(name="work", bufs=6))
        smax = ctx.enter_context(tc.tile_pool(name="smax", bufs=2))

        weT_sb = consts.tile([E, K], f32r)
        nc.gpsimd.dma_start(out=weT_sb, in_=weT[:])
        whT_sb = consts.tile([D, K], f32r)
        nc.gpsimd.dma_start(out=whT_sb, in_=whT[:])
        wvs_sb = consts.tile([K, 2 * S], f32r)
        nc.gpsimd.dma_start(out=wvs_sb, in_=wvs[:])
        batt_sb = consts.tile([K, 1], f32)
        nc.sync.dma_start(out=batt_sb, in_=batt[:])
        ones_tmp = consts.tile([S, S], f32)
        nc.vector.memset(ones_tmp, 1.0)
        ones_sb = consts.tile([S, S], f32r)
        nc.vector.tensor_copy(ones_sb, ones_tmp)

        hid_sb = consts.tile([D, B * FREE], f32r)
        for b in range(B):
            nc.gpsimd.dma_start(
                out=hid_sb[:, b * FREE : (b + 1) * FREE], in_=hid[b]
            )

        projh_sb = consts.tile([K, B, FREE], f32)
        for b in range(B):
            ph_ps = epsum.tile([K, FREE], f32, tag="e_ps", name="ph_ps")
            nc.tensor.matmul(
                ph_ps,
                lhsT=whT_sb,
                rhs=hid_sb[:, b * FREE : (b + 1) * FREE],
                start=True,
                stop=True,
            )
            nc.vector.tensor_copy(projh_sb[:, b, :], ph_ps)

        SG = S // 2
        for b in range(B):
            hslice = hid_sb[:, b * FREE : (b + 1) * FREE]
            sc_g = [
                spsum.tile([SG, FREE], f32, tag="scA", name="scA"),
                spsum.tile([SG, FREE], f32, tag="scB", name="scB", bufs=1),
            ]
            expv = smax.tile([S, FREE], f32r, tag="expv", name="expv")
            expf = smax.tile([S, FREE], f32, tag="expf", name="expf")
            chunks = [24, 24, 16] if b < B - 1 else [24, 16, 8, 8, 4, 2, 2]
            off = 0
            for csz in chunks:
                et = encp.tile([E, SCH * FREE], f32r, tag="et", name="et")
                nc.gpsimd.dma_start(
                    out=et[:, : csz * FREE],
                    in_=enc[b, :, off * FREE : (off + csz) * FREE],
                )
                for j in range(csz):
                    s = off + j
                    rhs = et[:, j * FREE : (j + 1) * FREE]
                    e_ps = epsum.tile([K, FREE], f32, tag="e_ps", name="e_ps")
                    pe_add = s % PE_ADD_EVERY == 0
                    nc.tensor.matmul(e_ps, lhsT=weT_sb, rhs=rhs,
                                     start=True, stop=not pe_add)
                    if pe_add:
                        nc.tensor.matmul(e_ps, lhsT=whT_sb, rhs=hslice,
                                         start=False, stop=True)
                    else:
                        nc.vector.tensor_add(
                            out=e_ps, in0=e_ps, in1=projh_sb[:, b, :]
                        )
                    th = work.tile([K, FREE], f32r, tag="th", name="th")
                    nc.scalar.activation(th, e_ps, AF.Tanh, bias=batt_sb)
                    g, sl = divmod(s, SG)
                    nc.tensor.matmul(
                        sc_g[g],
                        lhsT=wvs_sb[:, (SG - 1) - sl : (2 * SG - 1) - sl],
                        rhs=th,
                        start=(sl == 0),
                        stop=(sl == SG - 1),
                    )
                    if sl == SG - 1:
                        nc.scalar.activation(
                            expv[g * SG : (g + 1) * SG, :], sc_g[g], AF.Exp
                        )
                        nc.scalar.activation(
                            expf[g * SG : (g + 1) * SG, :], sc_g[g], AF.Exp
                        )
                off += csz
            sum_bc = spsum.tile([S, FREE], f32, tag="scA", name="sum_bc")
            nc.tensor.matmul(sum_bc, lhsT=ones_sb[:SG, :],
                             rhs=expv[:SG, :], start=True, stop=False)
            nc.tensor.matmul(sum_bc, lhsT=ones_sb[SG:, :],
                             rhs=expv[SG:, :], start=False, stop=True)
            rec = smax.tile([S, FREE], f32, tag="rec", name="rec")
            nc.vector.reciprocal(rec, sum_bc)
            ob = smax.tile([S, FREE], f32, tag="ob", name="ob")
            nc.vector.tensor_mul(ob, expf, rec)
            nc.sync.dma_start(out=out[b], in_=ob)
    nc.compile()
    return nc


def _get_bass():
    if "nc" not in _CACHE:
        _CACHE["nc"] = _build_bass()
    return _CACHE["nc"]


def kernel(hidden_state, encoder_outputs, W_att, b_att, W_v):
    from concourse.bass_utils import run_bass_kernel_spmd

    hidden_state = np.asarray(hidden_state, dtype=np.float32)
    encoder_outputs = np.asarray(encoder_outputs, dtype=np.float32)
    W_att = np.asarray(W_att, dtype=np.float32)
    b_att = np.asarray(b_att, dtype=np.float32)
    W_v = np.asarray(W_v, dtype=np.float32)

    weT = np.ascontiguousarray(W_att[:, D:].T)      # [E, K]
    whT = np.ascontiguousarray(W_att[:, :D].T)      # [D, K]
    batt = np.ascontiguousarray(b_att.reshape(K, 1))
    wvs = np.zeros((K, 2 * S), dtype=np.float32)
    wvs[:, S // 2 - 1] = W_v[0]

    in_maps = []
    for c in range(NCORES):
        h0 = c * HSH
        enc_c = np.ascontiguousarray(
            encoder_outputs[:, :, :, h0 : h0 + HSH, :]
        ).reshape(B, E, S * FREE)
        hid_c = np.ascontiguousarray(
            hidden_state[:, :, h0 : h0 + HSH, :]
        ).reshape(B, D, FREE)
        in_maps.append(
            {"enc": enc_c, "hid": hid_c, "weT": weT, "whT": whT,
             "batt": batt, "wvs": wvs}
        )

    nc = _get_bass()
    kwargs = dict(_CACHE.get("run_kwargs", {}))
    res = run_bass_kernel_spmd(nc, in_maps, core_ids=list(range(NCORES)), **kwargs)
    _CACHE["last_result"] = res
    shards = [r["out"].reshape(B, S, HSH, W) for r in res.results]
    return np.concatenate(shards, axis=2)


# revision 32
# speedup vs baseline: 1.0035x; 1.0035x over previous
# Trainium2 Bass kernel for nn_Attention_60464549593105.
#
# Math (per batch b, spatial point (h,w), seq s):
#   energy[k] = tanh( We @ enc[:,s] + Wh @ hidden + b_att )      (K=128)
#   score[s]  = W_v . energy
#   out[s]    = softmax_s(score)
#
# Strategy: shard the H axis across 8 cores (8 rows each) so softmax over
# seq is core-local (no collectives). Each core streams its 64 MiB slice of
# encoder_outputs once -> memory-bound; measured ~233 us vs the ~190 us
# HBM-stream floor (358 GB/s per core):
#   - all matmuls run in float32r (~1e-4/element rounding; measured 1.6e-4
#     rel on a 128-deep contraction vs bf16's 2.3e-3) which streams at
#     1 cycle/row vs fp32's LOW_HIGH 4 cycles/row. The fp32->fp32r rounding
#     is free everywhere: SWDGE DMA casts during the HBM load, and ACT
#     writes its tanh/exp outputs as fp32r directly.
#   - proj_e: PE matmul lhsT=We^T [E,K], rhs=enc chunk [E, 512] -> psum
#   - +proj_h: mostly a DVE add of a precomputed proj_h tile; 1 in 8 tiles
#     uses a second accumulating PE matmul to balance PE vs DVE occupancy
#   - tanh(+b_att): one ACT pass per s (psum -> sbuf fp32r), bias = b_att
#   - scores: matvec with a sliding-window masked W_v stationary operand so
#     per-s results accumulate directly into [32, 512] psum tiles
#     (partition = s mod 32) -> softmax-ready layout; two 32-row groups per
#     batch so the first exp can start halfway through the batch
#   - softmax over s: exp twice (fp32r copy for the sum matmul, fp32 copy
#     for the numerator; max-subtraction skipped -- |score| <= sum|W_v| ~ 5
#     so exp is safe in fp32), all-ones [32,64] matmuls = sum-over-
#     partitions broadcast to all 64 rows, DVE reciprocal + multiply.
#   - the last batch's DMA chunks taper off so the pipeline drains quickly.

import numpy as np

B, D, E, S, H, W = 4, 128, 128, 64, 64, 64
K = 128
NCORES = 8
HSH = H // NCORES          # h rows per core
FREE = HSH * W             # free-dim elements per (b, s) tile
SCH = 24                   # seq positions per enc DMA chunk (6 MiB per DMA)
ENC_BUFS = 3
PE_ADD_EVERY = 8           # s % PE_ADD_EVERY == 0 -> proj_h via PE, else DVE

_CACHE = {}


def _build_bass():
    import concourse.bacc as bacc
    import concourse.mybir as mybir
    import concourse.tile as tile
    from contextlib import ExitStack

    f32 = mybir.dt.float32
    f32r = mybir.dt.float32r
    AF = mybir.ActivationFunctionType

    nc = bacc.Bacc("TRN2", target_bir_lowering=False, debug=False)
    enc = nc.dram_tensor("enc", [B, E, S * FREE], f32, kind="ExternalInput")
    hid = nc.dram_tensor("hid", [B, D, FREE], f32, kind="ExternalInput")
    weT = nc.dram_tensor("weT", [E, K], f32, kind="ExternalInput")
    whT = nc.dram_tensor("whT", [D, K], f32, kind="ExternalInput")
    batt = nc.dram_tensor("batt", [K, 1], f32, kind="ExternalInput")
    wvs = nc.dram_tensor("wvs", [K, 2 * S], f32, kind="ExternalInput")
    out = nc.dram_tensor("out", [B, S, FREE], f32, kind="ExternalOutput")

    with tile.TileContext(nc) as tc, ExitStack() as ctx:
        consts = ctx.enter_context(tc.tile_pool(name="consts", bufs=1))
        encp = ctx.enter_context(tc.tile_pool(name="encp", bufs=ENC_BUFS))
        epsum = ctx.enter_context(tc.tile_pool(name="epsum", bufs=5, space="PSUM"))
        spsum = ctx.enter_context(tc.tile_pool(name="spsum", bufs=2, space="PSUM"))
        work = ctx.enter_context(tc.tile_pool(name="work", bufs=6))
        smax = ctx.enter_context(tc.tile_pool(name="smax", bufs=2))

        weT_sb = consts.tile([E, K], f32r)
        nc.gpsimd.dma_start(out=weT_sb, in_=weT[:])
        whT_sb = consts.tile([D, K], f32r)
        nc.gpsimd.dma_start(out=whT_sb, in_=whT[:])
        wvs_sb = consts.tile([K, 2 * S], f32r)
        nc.gpsimd.dma_start(out=wvs_sb, in_=wvs[:])
        batt_sb = consts.tile([K, 1], f32)
        nc.sync.dma_start(out=batt_sb, in_=batt[:])
        ones_tmp = consts.tile([S, S], f32)
        nc.vector.memset(ones_tmp, 1.0)
        ones_sb = consts.tile([S, S], f32r)
        nc.vector.tensor_copy(ones_sb, ones_tmp)

        hid_sb = consts.tile([D, B * FREE], f32r)
        for b in range(B):
            nc.gpsimd.dma_start(
                out=hid_sb[:, b * FREE : (b + 1) * FREE], in_=hid[b]
            )

        projh_sb = consts.tile([K, B, FREE], f32)
        for b in range(B):
            ph_ps = epsum.tile([K, FREE], f32, tag="e_ps", name="ph_ps")
            nc.tensor.matmul(
                ph_ps,
                lhsT=whT_sb,
                rhs=hid_sb[:, b * FREE : (b + 1) * FREE],
                start=True,
                stop=True,
            )
            nc.vector.tensor_copy(projh_sb[:, b, :], ph_ps)

        SG = S // 2
        for b in range(B):
            hslice = hid_sb[:, b * FREE : (b + 1) * FREE]
            sc_g = [
                spsum.tile([SG, FREE], f32, tag="scA", name="scA"),
                spsum.tile([SG, FREE], f32, tag="scB", name="scB", bufs=1),
            ]
            expv = smax.tile([S, FREE], f32r, tag="expv", name="expv")
            expf = smax.tile([S, FREE], f32, tag="expf", name="expf")
            if b == 0:        # ramp-up: compute starts after the first 0.5 MiB
                chunks = [2, 2, 4, 8, 24, 24]
            elif b < B - 1:
                chunks = [24, 24, 16]
            else:             # taper: quick pipeline drain
                chunks = [24, 16, 8, 8, 4, 2, 2]
            off = 0
            for csz in chunks:
                et = encp.tile([E, SCH * FREE], f32r, tag="et", name="et")
                nc.gpsimd.dma_start(
                    out=et[:, : csz * FREE],
                    in_=enc[b, :, off * FREE : (off + csz) * FREE],
                )
                for j in range(csz):
                    s = off + j
                    rhs = et[:, j * FREE : (j + 1) * FREE]
                    e_ps = epsum.tile([K, FREE], f32, tag="e_ps", name="e_ps")
                    pe_add = s % PE_ADD_EVERY == 0
                    nc.tensor.matmul(e_ps, lhsT=weT_sb, rhs=rhs,
                                     start=True, stop=not pe_add)
                    if pe_add:
                        nc.tensor.matmul(e_ps, lhsT=whT_sb, rhs=hslice,
                                         start=False, stop=True)
                    else:
                        nc.vector.tensor_add(
                            out=e_ps, in0=e_ps, in1=projh_sb[:, b, :]
                        )
                    th = work.tile([K, FREE], f32r, tag="th", name="th")
                    nc.scalar.activation(th, e_ps, AF.Tanh, bias=batt_sb)
                    g, sl = divmod(s, SG)
                    nc.tensor.matmul(
                        sc_g[g],
                        lhsT=wvs_sb[:, (SG - 1) - sl : (2 * SG - 1) - sl],
                        rhs=th,
                        start=(sl == 0),
                        stop=(sl == SG - 1),
                    )
                    if sl == SG - 1:
                        nc.scalar.activation(
                            expv[g * SG : (g + 1) * SG, :], sc_g[g], AF.Exp
                        )
                        nc.scalar.activation(
                            expf[g * SG : (g + 1) * SG, :], sc_g[g], AF.Exp
                        )
                off += csz
            sum_bc = spsum.tile([S, FREE], f32, tag="scA", name="sum_bc")
            nc.tensor.matmul(sum_bc, lhsT=ones_sb[:SG, :],
                             rhs=expv[:SG, :], start=True, stop=False)
            nc.tensor.matmul(sum_bc, lhsT=ones_sb[SG:, :],
                             rhs=expv[SG:, :], start=False, stop=True)
            rec = smax.tile([S, FREE], f32, tag="rec", name="rec")
            nc.vector.reciprocal(rec, sum_bc)
            ob = smax.tile([S, FREE], f32, tag="ob", name="ob")
            nc.vector.tensor_mul(ob, expf, rec)
            nc.sync.dma_start(out=out[b], in_=ob)
    nc.compile()
    return nc


def _get_bass():
    if "nc" not in _CACHE:
        _CACHE["nc"] = _build_bass()
    return _CACHE["nc"]


def kernel(hidden_state, encoder_outputs, W_att, b_att, W_v):
    from concourse.bass_utils import run_bass_kernel_spmd

    hidden_state = np.asarray(hidden_state, dtype=np.float32)
    encoder_outputs = np.asarray(encoder_outputs, dtype=np.float32)
    W_att = np.asarray(W_att, dtype=np.float32)
    b_att = np.asarray(b_att, dtype=np.float32)
    W_v = np.asarray(W_v, dtype=np.float32)

    weT = np.ascontiguousarray(W_att[:, D:].T)      # [E, K]
    whT = np.ascontiguousarray(W_att[:, :D].T)      # [D, K]
    batt = np.ascontiguousarray(b_att.reshape(K, 1))
    wvs = np.zeros((K, 2 * S), dtype=np.float32)
    wvs[:, S // 2 - 1] = W_v[0]

    in_maps = []
    for c in range(NCORES):
        h0 = c * HSH
        enc_c = np.ascontiguousarray(
            encoder_outputs[:, :, :, h0 : h0 + HSH, :]
        ).reshape(B, E, S * FREE)
        hid_c = np.ascontiguousarray(
            hidden_state[:, :, h0 : h0 + HSH, :]
        ).reshape(B, D, FREE)
        in_maps.append(
            {"enc": enc_c, "hid": hid_c, "weT": weT, "whT": whT,
             "batt": batt, "wvs": wvs}
        )

    nc = _get_bass()
    kwargs = dict(_CACHE.get("run_kwargs", {}))
    res = run_bass_kernel_spmd(nc, in_maps, core_ids=list(range(NCORES)), **kwargs)
    _CACHE["last_result"] = res
    shards = [r["out"].reshape(B, S, HSH, W) for r in res.results]
    return np.concatenate(shards, axis=2)


# revision 33
# speedup vs baseline: 1.0063x; 1.0028x over previous
# Trainium2 Bass kernel for nn_Attention_60464549593105.
#
# Math (per batch b, spatial point (h,w), seq s):
#   energy[k] = tanh( We @ enc[:,s] + Wh @ hidden + b_att )      (K=128)
#   score[s]  = W_v . energy
#   out[s]    = softmax_s(score)
#
# Strategy: shard the H axis across 8 cores (8 rows each) so softmax over
# seq is core-local (no collectives). Each core streams its 64 MiB slice of
# encoder_outputs once -> memory-bound; measured ~233 us vs the ~190 us
# HBM-stream floor (358 GB/s per core):
#   - all matmuls run in float32r (~1e-4/element rounding; measured 1.6e-4
#     rel on a 128-deep contraction vs bf16's 2.3e-3) which streams at
#     1 cycle/row vs fp32's LOW_HIGH 4 cycles/row. The fp32->fp32r rounding
#     is free everywhere: SWDGE DMA casts during the HBM load, and ACT
#     writes its tanh/exp outputs as fp32r directly.
#   - proj_e: PE matmul lhsT=We^T [E,K], rhs=enc chunk [E, 512] -> psum
#   - +proj_h: mostly a DVE add of a precomputed proj_h tile; 1 in 8 tiles
#     uses a second accumulating PE matmul to balance PE vs DVE occupancy
#   - tanh(+b_att): one ACT pass per s (psum -> sbuf fp32r), bias = b_att
#   - scores: matvec with a sliding-window masked W_v stationary operand so
#     per-s results accumulate directly into [32, 512] psum tiles
#     (partition = s mod 32) -> softmax-ready layout; two 32-row groups per
#     batch so the first exp can start halfway through the batch
#   - softmax over s: exp twice (fp32r copy for the sum matmul, fp32 copy
#     for the numerator; max-subtraction skipped -- |score| <= sum|W_v| ~ 5
#     so exp is safe in fp32), all-ones [32,64] matmuls = sum-over-
#     partitions broadcast to all 64 rows, DVE reciprocal + multiply.
#   - the last batch's DMA chunks taper off so the pipeline drains quickly.

import numpy as np

B, D, E, S, H, W = 4, 128, 128, 64, 64, 64
K = 128
NCORES = 8
HSH = H // NCORES          # h rows per core
FREE = HSH * W             # free-dim elements per (b, s) tile
SCH = 24                   # seq positions per enc DMA chunk (6 MiB per DMA)
ENC_BUFS = 3
PE_ADD_EVERY = 8           # s % PE_ADD_EVERY == 0 -> proj_h via PE, else DVE

_CACHE = {}


def _build_bass():
    import concourse.bacc as bacc
    import concourse.mybir as mybir
    import concourse.tile as tile
    from contextlib import ExitStack

    f32 = mybir.dt.float32
    f32r = mybir.dt.float32r
    AF = mybir.ActivationFunctionType

    nc = bacc.Bacc("TRN2", target_bir_lowering=False, debug=False)
    enc = nc.dram_tensor("enc", [B, E, S * FREE], f32, kind="ExternalInput")
    hid = nc.dram_tensor("hid", [B, D, FREE], f32, kind="ExternalInput")
    weT = nc.dram_tensor("weT", [E, K], f32, kind="ExternalInput")
    whT = nc.dram_tensor("whT", [D, K], f32, kind="ExternalInput")
    batt = nc.dram_tensor("batt", [K, 1], f32, kind="ExternalInput")
    wvs = nc.dram_tensor("wvs", [K, 2 * S], f32, kind="ExternalInput")
    out = nc.dram_tensor("out", [B, S, FREE], f32, kind="ExternalOutput")

    with tile.TileContext(nc) as tc, ExitStack() as ctx:
        consts = ctx.enter_context(tc.tile_pool(name="consts", bufs=1))
        encp = ctx.enter_context(tc.tile_pool(name="encp", bufs=ENC_BUFS))
        epsum = ctx.enter_context(tc.tile_pool(name="epsum", bufs=5, space="PSUM"))
        spsum = ctx.enter_context(tc.tile_pool(name="spsum", bufs=2, space="PSUM"))
        work = ctx.enter_context(tc.tile_pool(name="work", bufs=6))
        smax = ctx.enter_context(tc.tile_pool(name="smax", bufs=2))

        weT_sb = consts.tile([E, K], f32r)
        nc.gpsimd.dma_start(out=weT_sb, in_=weT[:])
        whT_sb = consts.tile([D, K], f32r)
        nc.gpsimd.dma_start(out=whT_sb, in_=whT[:])
        wvs_sb = consts.tile([K, 2 * S], f32r)
        nc.gpsimd.dma_start(out=wvs_sb, in_=wvs[:])
        batt_sb = consts.tile([K, 1], f32)
        nc.sync.dma_start(out=batt_sb, in_=batt[:])
        ones_tmp = consts.tile([S, S], f32)
        nc.vector.memset(ones_tmp, 1.0)
        ones_sb = consts.tile([S, S], f32r)
        nc.vector.tensor_copy(ones_sb, ones_tmp)

        hid_sb = consts.tile([D, B * FREE], f32r)
        for b in range(B):
            nc.gpsimd.dma_start(
                out=hid_sb[:, b * FREE : (b + 1) * FREE], in_=hid[b]
            )

        projh_sb = consts.tile([K, B, FREE], f32)
        for b in range(B):
            ph_ps = epsum.tile([K, FREE], f32, tag="e_ps", name="ph_ps")
            nc.tensor.matmul(
                ph_ps,
                lhsT=whT_sb,
                rhs=hid_sb[:, b * FREE : (b + 1) * FREE],
                start=True,
                stop=True,
            )
            nc.vector.tensor_copy(projh_sb[:, b, :], ph_ps)

        SG = S // 2
        for b in range(B):
            hslice = hid_sb[:, b * FREE : (b + 1) * FREE]
            sc_g = [
                spsum.tile([SG, FREE], f32, tag="scA", name="scA"),
                spsum.tile([SG, FREE], f32, tag="scB", name="scB", bufs=1),
            ]
            expv = smax.tile([S, FREE], f32r, tag="expv", name="expv")
            expf = smax.tile([S, FREE], f32, tag="expf", name="expf")
            if b == 0:        # ramp-up: compute starts after the first 0.5 MiB
                chunks = [2, 2, 4, 8, 24, 24]
            elif b < B - 1:
                chunks = [24, 24, 16]
            else:             # taper: quick pipeline drain
                chunks = [24, 16, 8, 8, 4, 2, 2]
            off = 0
            for csz in chunks:
                et = encp.tile([E, SCH * FREE], f32r, tag="et", name="et")
                nc.gpsimd.dma_start(
                    out=et[:, : csz * FREE],
                    in_=enc[b, :, off * FREE : (off + csz) * FREE],
                )
                for j in range(csz):
                    s = off + j
                    rhs = et[:, j * FREE : (j + 1) * FREE]
                    e_ps = epsum.tile([K, FREE], f32, tag="e_ps", name="e_ps")
                    pe_add = s % PE_ADD_EVERY == 0
                    nc.tensor.matmul(e_ps, lhsT=weT_sb, rhs=rhs,
                                     start=True, stop=not pe_add)
                    if pe_add:
                        nc.tensor.matmul(e_ps, lhsT=whT_sb, rhs=hslice,
                                         start=False, stop=True)
                    else:
                        nc.vector.tensor_add(
                            out=e_ps, in0=e_ps, in1=projh_sb[:, b, :]
                        )
                    th = work.tile([K, FREE], f32r, tag="th", name="th")
                    nc.scalar.activation(th, e_ps, AF.Tanh, bias=batt_sb)
                    g, sl = divmod(s, SG)
                    nc.tensor.matmul(
                        sc_g[g],
                        lhsT=wvs_sb[:, (SG - 1) - sl : (2 * SG - 1) - sl],
                        rhs=th,
                        start=(sl == 0),
                        stop=(sl == SG - 1),
                    )
                    if sl == SG - 1:
                        nc.scalar.activation(
                            expv[g * SG : (g + 1) * SG, :], sc_g[g], AF.Exp
                        )
                        nc.scalar.activation(
                            expf[g * SG : (g + 1) * SG, :], sc_g[g], AF.Exp
                        )
                off += csz
            sum_bc = spsum.tile([S, FREE], f32, tag="scA", name="sum_bc")
            nc.tensor.matmul(sum_bc, lhsT=ones_sb[:SG, :],
                             rhs=expv[:SG, :], start=True, stop=False)
            nc.tensor.matmul(sum_bc, lhsT=ones_sb[SG:, :],
                             rhs=expv[SG:, :], start=False, stop=True)
            rec = smax.tile([S, FREE], f32, tag="rec", name="rec")
            rscr = smax.tile([S, FREE], f32, tag="rscr", name="rscr")
            nc.vector.reciprocal_approx_accurate(out=rec, in_=sum_bc, scratch=rscr)
            ob = smax.tile([S, FREE], f32, tag="ob", name="ob")
            nc.gpsimd.tensor_mul(ob, expf, rec)
            nc.sync.dma_start(out=out[b], in_=ob)
    nc.compile()
    return nc


def _get_bass():
    if "nc" not in _CACHE:
        _CACHE["nc"] = _build_bass()
    return _CACHE["nc"]


def kernel(hidden_state, encoder_outputs, W_att, b_att, W_v):
    from concourse.bass_utils import run_bass_kernel_spmd

    hidden_state = np.asarray(hidden_state, dtype=np.float32)
    encoder_outputs = np.asarray(encoder_outputs, dtype=np.float32)
    W_att = np.asarray(W_att, dtype=np.float32)
    b_att = np.asarray(b_att, dtype=np.float32)
    W_v = np.asarray(W_v, dtype=np.float32)

    weT = np.ascontiguousarray(W_att[:, D:].T)      # [E, K]
    whT = np.ascontiguousarray(W_att[:, :D].T)      # [D, K]
    batt = np.ascontiguousarray(b_att.reshape(K, 1))
    wvs = np.zeros((K, 2 * S), dtype=np.float32)
    wvs[:, S // 2 - 1] = W_v[0]

    in_maps = []
    for c in range(NCORES):
        h0 = c * HSH
        enc_c = np.ascontiguousarray(
            encoder_outputs[:, :, :, h0 : h0 + HSH, :]
        ).reshape(B, E, S * FREE)
        hid_c = np.ascontiguousarray(
            hidden_state[:, :, h0 : h0 + HSH, :]
        ).reshape(B, D, FREE)
        in_maps.append(
            {"enc": enc_c, "hid": hid_c, "weT": weT, "whT": whT,
             "batt": batt, "wvs": wvs}
        )

    nc = _get_bass()
    kwargs = dict(_CACHE.get("run_kwargs", {}))
    res = run_bass_kernel_spmd(nc, in_maps, core_ids=list(range(NCORES)), **kwargs)
    _CACHE["last_result"] = res
    shards = [r["out"].reshape(B, S, HSH, W) for r in res.results]
    return np.concatenate(shards, axis=2)
